# revision 1
# baseline (speedup 1.0000x reference)
"""Causal linear attention (elu+1 feature map) on 8 Trainium2 NeuronCores.

Full inputs (n=2, l=2048, h=8, d=64) fp32 are sharded over the 16 (n,h)
head-sequences: core i handles pairs (2i, 2i+1). Each core runs a two-level
chunked scan (chunk C=128, state stride 2 chunks):

  [AT(c) | CROSS] = Kf_c @ [Qf_c | Qf_{c+1}]^T    (one matmul, both pairs)
  AT(c+1)         = Kf_{c+1} @ Qf_{c+1}^T
  out(c)   = ATm(c)^T @ Vaug_c + Qf_c @ S                    ; out /= denom
  out(c+1) = ATm(c+1)^T @ Vaug_{c+1} + CROSS^T @ Vaug_c + Qf_{c+1} @ S
  S       += Kf_c^T @ Vaug_c + Kf_{c+1}^T @ Vaug_{c+1}   (PSUM fp32)

The 2-chunk state stride halves the serial PE->snapshot->PE chain.
Feature map: elu(x)+1 = min(exp(x), max(x+1,1)): exp on ScalarE,
clamp + min on DVE.

qfb layout trick: Q features live in a pair-block structure
qfb[(p',d), 1024p + 128c + i], nonzero only for p'==p (off-blocks zeroed
once; tiles are persistent so the zeros survive). One dense kfT stationary
times this blocked moving operand yields both pairs' AT in one matmul, and
blocked stationaries pull each pair's inter-chunk term from the
(garbage-tolerant) S state. All matmuls keep base-partition-0 operands: PE
quadrant (tile_position) matmuls hang TRN2 when pipelined, as do DVE reads
of the PSUM bank the PE is accumulating S into (the snapshot runs on
ScalarE for that reason).

PSUM accumulation banks get their single start=True from a K=1 all-zeros
matmul; real matmuls all accumulate (start=False) — order-robust, since a
start=True invalidates its whole 2KB PSUM bank.

Host layouts (fp16, all DMAs contiguous):
  qT, kT: (128, 2048)  [(64p + d), (128c + i)]   (host-transposed)
  k,  v : (128, 2048)  [i, 128c + 64p + d]       (natural)
  out   : (128, 2048) fp32, same indexing as k/v.
"""
import numpy as np
from contextlib import ExitStack

import concourse.bacc as bacc
import concourse.bass as bass
import concourse.tile as tile
from concourse import mybir
from concourse.bass_utils import run_bass_kernel_spmd

N, L, H, D = 2, 2048, 8, 64
C = 128                 # chunk length
NCH = L // C            # 16 chunks
GROUP = 8               # chunks per fmap/DMA group
NGRP = NCH // GROUP
PAIRS = 2
W = NCH * PAIRS * D     # 2048
GW = GROUP * PAIRS * D  # 1024 natural cols per group
TW = GROUP * C          # 1024 transposed cols per group
BW = PAIRS * TW         # 2048 blocked cols per group (pair-major)
VW = GROUP * PAIRS * (D + 1)   # 1040 v cols per group (with ones col)
SW = PAIRS * (D + 1)    # 130: S cols [S_p0 | ksum_p0 | S_p1 | ksum_p1]
ATW = 6 * C             # at tile: [ATc p0|CROSS p0|ATc p1|CROSS p1|ATc1 p0p1]

f16 = mybir.dt.float16
f32 = mybir.dt.float32
AF = mybir.ActivationFunctionType
OP = mybir.AluOpType


def _fmap(nc, pool, src, width, tag):
    """f = min(exp(x), max(x+1,1)): exp on ACT, clamp + min on DVE."""
    e = pool.tile([C, width], f16, tag=f"e_{tag}")
    t = pool.tile([C, width], f16, tag=f"t_{tag}")
    nc.scalar.activation(e, src, AF.Exp)
    nc.vector.tensor_scalar(out=t, in0=src, scalar1=1.0, scalar2=1.0,
                            op0=OP.add, op1=OP.max)
    return e, t


def build_kernel():
    nc = bacc.Bacc("TRN2", target_bir_lowering=False, debug=False, num_devices=8)
    qT_d = nc.dram_tensor("qT", (C, W), f16, kind="ExternalInput").ap()
    kT_d = nc.dram_tensor("kT", (C, W), f16, kind="ExternalInput").ap()
    k_d = nc.dram_tensor("k", (C, W), f16, kind="ExternalInput").ap()
    v_d = nc.dram_tensor("v", (C, W), f16, kind="ExternalInput").ap()
    o_d = nc.dram_tensor("o", (C, W), f32, kind="ExternalOutput").ap()

    with tile.TileContext(nc) as tc, ExitStack() as ctx:
        consts = ctx.enter_context(tc.tile_pool(name="consts", bufs=1))
        io_pool = ctx.enter_context(tc.tile_pool(name="io", bufs=2))
        fm_pool = ctx.enter_context(tc.tile_pool(name="fm", bufs=2))
        sm_pool = ctx.enter_context(tc.tile_pool(name="sm", bufs=3))
        at_psum = ctx.enter_context(tc.tile_pool(name="at", bufs=2, space="PSUM"))
        out_psum = ctx.enter_context(tc.tile_pool(name="out", bufs=3, space="PSUM"))
        s_psum = ctx.enter_context(tc.tile_pool(name="sp", bufs=1, space="PSUM"))

        zeros = consts.tile([1, 4 * C], f16)
        nc.gpsimd.memset(zeros, 0.0)

        # mask blocks: [tri, ones, tri, ones, tri, tri] (128 cols each)
        maskT = consts.tile([C, ATW], f32)
        m6 = maskT.rearrange("j (b i) -> j b i", b=6)
        nc.gpsimd.memset(maskT, 0.0)
        for blk in ((0, 1), (2, 3), (4, 6)):
            nc.gpsimd.affine_select(
                out=m6[:, blk[0]:blk[1]], in_=m6[:, blk[0]:blk[1]],
                compare_op=OP.is_gt, fill=1.0,
                base=0, pattern=[[0, blk[1] - blk[0]], [-1, C]],
                channel_multiplier=1,
            )
        nc.gpsimd.memset(m6[:, 1:2], 1.0)
        nc.gpsimd.memset(m6[:, 3:4], 1.0)

        # persistent running state (off-pair blocks accumulate unread garbage)
        S_ps = s_psum.tile([C, SW], f32)
        nc.tensor.matmul(S_ps, zeros[:, 0:C], zeros[:, 0:SW],
                         start=True, stop=False, skip_group_check=True)

        # persistent double-buffered tiles: qfb off-blocks and the v ones
        # columns are written once and never touched by per-group writes
        qfbs, vgs = [], []
        for b in range(2):
            qfb = consts.tile([C, BW], f16, tag=f"qfb{b}")
            nc.gpsimd.memset(qfb, 0.0)
            qfbs.append(qfb)
            v_g = consts.tile([C, VW], f16, tag=f"v_g{b}")
            v4 = v_g.rearrange("i (j b x) -> i j b x", j=GROUP, b=PAIRS)
            nc.gpsimd.memset(v4[:, :, :, D:D + 1], 1.0)
            vgs.append(v_g)

        for g in range(NGRP):
            gsl = slice(g * GW, (g + 1) * GW)
            tsl = slice(g * TW, (g + 1) * TW)

            qT_g = io_pool.tile([C, TW], f16, tag="qT_g")
            kT_g = io_pool.tile([C, TW], f16, tag="kT_g")
            k_g = io_pool.tile([C, GW], f16, tag="k_g")
            v_g = vgs[g % 2]
            v4 = v_g.rearrange("i (j b x) -> i j b x", j=GROUP, b=PAIRS)
            nc.sync.dma_start(qT_g, qT_d[:, tsl])
            nc.sync.dma_start(kT_g, kT_d[:, tsl])
            nc.sync.dma_start(k_g, k_d[:, gsl])
            nc.sync.dma_start(
                v4[:, :, :, 0:D],
                v_d[:, gsl].rearrange("i (j b x) -> i j b x", j=GROUP, b=PAIRS),
            )

            # feature maps
            e_q, t_q = _fmap(nc, fm_pool, qT_g, TW, "q")
            qfb = qfbs[g % 2]
            for p in range(PAIRS):
                rows = slice(p * D, (p + 1) * D)
                nc.vector.tensor_tensor(
                    out=qfb[rows, p * TW:(p + 1) * TW],
                    in0=e_q[rows], in1=t_q[rows], op=OP.min)
            qfb3 = qfb.rearrange("r (p x) -> r p x", p=PAIRS)

            e_kT, t_kT = _fmap(nc, fm_pool, kT_g, TW, "kT")
            kfT = fm_pool.tile([C, TW], f16, tag="kfT")
            nc.vector.tensor_tensor(out=kfT, in0=e_kT, in1=t_kT, op=OP.min)

            e_k, t_k = _fmap(nc, fm_pool, k_g, GW, "k")
            kf = fm_pool.tile([C, GW], f16, tag="kf")
            nc.vector.tensor_tensor(out=kf, in0=e_k, in1=t_k, op=OP.min)

            stage = io_pool.tile([C, GW], f32, tag="stage")

            for jj in range(GROUP // 2):    # two chunks per scan step
                j0, j1 = 2 * jj, 2 * jj + 1
                c0 = g * GROUP + j0
                t0 = slice(j0 * C, (j0 + 1) * C)
                t1 = slice(j1 * C, (j1 + 1) * C)
                t01 = slice(j0 * C, (j1 + 1) * C)

                at_ps = at_psum.tile([C, ATW], f32, tag="at")
                # [AT(c) | CROSS] both pairs in one matmul (shared stationary)
                nc.tensor.matmul(at_ps[:, 0:4 * C], kfT[:, t0],
                                 qfb3[:, :, t01], start=True, stop=True)
                nc.tensor.matmul(at_ps[:, 4 * C:6 * C], kfT[:, t1],
                                 qfb3[:, :, t1], start=True, stop=True)

                out_ps = out_psum.tile([C, 2 * SW], f32, tag="out")
                nc.tensor.matmul(out_ps, zeros[:, 0:C], zeros[:, 0:2 * SW],
                                 start=True, stop=False, skip_group_check=True)

                # state snapshot (state through chunk c0-1); ScalarE on purpose
                if c0 > 0:
                    S_sb = sm_pool.tile([C, SW], f16, tag="s_sb")
                    nc.scalar.copy(S_sb, S_ps)
                    for dj, tx in ((0, t0), (1, t1)):
                        for p in range(PAIRS):
                            vs = slice(p * (D + 1), (p + 1) * (D + 1))
                            nc.tensor.matmul(
                                out_ps[:, dj * SW + vs.start:dj * SW + vs.stop],
                                qfb[:, p * TW + tx.start:p * TW + tx.stop],
                                S_sb[:, vs],
                                start=False, stop=False, skip_group_check=True)

                # state updates, both chunks (after the snapshot read)
                for j, c in ((j0, c0), (j1, c0 + 1)):
                    if c < NCH - 1:
                        nc.tensor.matmul(
                            S_ps,
                            kf[:, j * PAIRS * D:(j + 1) * PAIRS * D],
                            v_g[:, j * SW:(j + 1) * SW],
                            start=False, stop=(c == NCH - 2),
                            skip_group_check=True)

                # mask ATs + copy CROSS in one DVE op
                atm = sm_pool.tile([C, ATW], f16, tag="atm")
                nc.vector.tensor_mul(atm, at_ps, maskT)

                # intra-chunk + cross contributions
                for p in range(PAIRS):
                    vs0 = slice(p * (D + 1), (p + 1) * (D + 1))
                    nc.tensor.matmul(        # out1(c0)
                        out_ps[:, vs0],
                        atm[:, 2 * p * C:(2 * p + 1) * C], v4[:, j0, p, :],
                        start=False, stop=False, skip_group_check=True)
                    nc.tensor.matmul(        # cross -> c1
                        out_ps[:, SW + vs0.start:SW + vs0.stop],
                        atm[:, (2 * p + 1) * C:(2 * p + 2) * C], v4[:, j0, p, :],
                        start=False, stop=False, skip_group_check=True)
                    nc.tensor.matmul(        # out1(c1)
                        out_ps[:, SW + vs0.start:SW + vs0.stop],
                        atm[:, (4 + p) * C:(5 + p) * C], v4[:, j1, p, :],
                        start=False, stop=(p == PAIRS - 1),
                        skip_group_check=True)

                # out = num * (1/den) for both chunks+pairs
                o5 = out_ps.rearrange("i (a b x) -> i a b x", a=2, b=PAIRS)
                recip = sm_pool.tile([C, 2, PAIRS, 1], f32, tag="recip")
                nc.vector.reciprocal(recip, o5[:, :, :, D:D + 1])
                rec_b = bass.AP(
                    tensor=recip.tensor, offset=recip.offset,
                    ap=[list(recip.ap[0]), list(recip.ap[1]),
                        list(recip.ap[2]), [0, D]],
                )
                st4 = stage.rearrange(
                    "i (j b x) -> i j b x", j=GROUP, b=PAIRS)[:, 2 * jj:2 * jj + 2]
                nc.vector.tensor_tensor(
                    out=st4, in0=o5[:, :, :, 0:D], in1=rec_b, op=OP.mult)

            nc.sync.dma_start(o_d[:, gsl], stage)

    nc.compile()
    return nc


_nc_cache = None


def _get_nc():
    global _nc_cache
    if _nc_cache is None:
        _nc_cache = build_kernel()
    return _nc_cache


def _core_pairs(x, core):
    flat = x.transpose(0, 2, 1, 3).reshape(N * H, L, D)
    return flat[2 * core:2 * core + 2]          # (2, L, D) fp32


def _nat_layout(xc):
    # (2, L, D) -> (128, 2048) [i, 128c + 64p + d]
    return np.ascontiguousarray(
        xc.reshape(PAIRS, NCH, C, D).transpose(2, 1, 0, 3).reshape(C, W)
    ).astype(np.float16)


def _t_layout(xc):
    # (2, L, D) -> (128, 2048) [(64p + d), (128c + i)]
    return np.ascontiguousarray(
        xc.reshape(PAIRS, NCH, C, D).transpose(0, 3, 1, 2).reshape(C, W)
    ).astype(np.float16)


def make_in_maps(queries, keys, values):
    in_maps = []
    for core in range(8):
        qc = _core_pairs(queries, core)
        kc = _core_pairs(keys, core)
        vc = _core_pairs(values, core)
        in_maps.append({
            "qT": _t_layout(qc),
            "kT": _t_layout(kc),
            "k": _nat_layout(kc),
            "v": _nat_layout(vc),
        })
    return in_maps


def kernel(queries, keys, values):
    nc = _get_nc()
    in_maps = make_in_maps(queries, keys, values)
    res = run_bass_kernel_spmd(nc, in_maps, core_ids=list(range(8)))
    out = np.zeros((N, L, H, D), np.float32)
    for core in range(8):
        oc = res.results[core]["o"].reshape(C, NCH, PAIRS, D)
        oc = oc.transpose(2, 1, 0, 3).reshape(PAIRS, L, D)
        for p in range(PAIRS):
            flat = 2 * core + p
            out[flat // H, :, flat % H, :] = oc[p]
    return out



# revision 4
# speedup vs baseline: 1.0704x; 1.0704x over previous
"""Causal linear attention (elu+1 feature map) on 8 Trainium2 NeuronCores.

Full inputs (n=2, l=2048, h=8, d=64) fp32 are sharded over the 16 (n,h)
head-sequences: core i handles pairs (2i, 2i+1). Single-pass design (all 16
chunks of C=128 in one group), stride-2 scan (2 chunks per step).

Per step (chunks c0=2s, c1=2s+1):
  AT(c0), AT(c1): one matmul each (stationary kfT chunk, moving blocked qfb
  both pairs) into one PSUM bank [128, 512] = [ATc0p0|ATc0p1|ATc1p0|ATc1p1].
  Causal masking = ONE DVE multiply with a broadcast [128,128] tri mask
  (stride-0 AP over the 4 blocks).

  Cross-chunk term (c0 -> c1) never materializes a 128x128 AT block: the
  rank-64 factorization G_c0 = Kf_c0^T @ Vaug_c0 (the chunk's state
  increment, one extra matmul into its own PSUM bank + one 130-col ACT copy)
  gives cross = Qf_c1 @ G_c0 via the same stationary as Q@S.

  out_ps [128, 260] accumulates Q@S_snap + Q@G + ATm^T V per (chunk, pair),
  with a trailing denominator column from vaug's baked-in ones column.
  First matmul into each PSUM bank uses start=True (clears the bank; no
  zeros-init matmuls).

  Normalization is deferred: numerators are evacuated PSUM->SBUF stage by
  ScalarE (Copy), reciprocals collected per step into a [128, 32] tile by
  DVE, and every 2 steps one in-place f16 DVE multiply (recip broadcast via
  stride-0 AP) normalizes 512 staged columns, followed by the output DMA.

  The feature map f = min(exp(x), max(x+1, 1)) = elu(x)+1 runs once on qT
  and once on kT (exp on ScalarE, the rest on DVE); the natural-layout kf
  needed as the G/S stationary comes from an SBUF->SBUF DMA xbar transpose
  of the *feature-mapped* kfT (3D out AP [i, c, pd]), and the blocked qfb
  (off-pair blocks zero, for the shared-stationary AT trick) is assembled
  by two partition-sliced SBUF->SBUF DMAs from the dense qf plus one-time
  GpSimd memsets of the off-blocks.

Host layouts (fp16, all DMAs contiguous):
  qT, kT: (128, 2048)  [(64p + d), (128c + i)]   (host-transposed)
  v     : (128, 2080)  [i, (c, p, dv)] dv=65, ones baked in at dv=64
  o     : (128, 2048)  [i', (c, p, d)] fp16
"""
import numpy as np
from contextlib import ExitStack

import concourse.bacc as bacc
import concourse.bass as bass
import concourse.tile as tile
from concourse import mybir
from concourse.bass_utils import run_bass_kernel_spmd

N, L, H, D = 2, 2048, 8, 64
C = 128                 # chunk length
NCH = L // C            # 16 chunks
PAIRS = 2
W = NCH * PAIRS * D     # 2048
DV = D + 1              # 65: value cols + denominator ones col
VW = NCH * PAIRS * DV   # 2080
SW = PAIRS * DV         # 130

f16 = mybir.dt.float16
f32 = mybir.dt.float32
AF = mybir.ActivationFunctionType
OP = mybir.AluOpType


def build_kernel():
    nc = bacc.Bacc("TRN2", target_bir_lowering=False, debug=False, num_devices=8)
    qT_d = nc.dram_tensor("qT", (C, W), f16, kind="ExternalInput").ap()
    kT_d = nc.dram_tensor("kT", (C, W), f16, kind="ExternalInput").ap()
    v_d = nc.dram_tensor("v", (C, VW), f16, kind="ExternalInput").ap()
    o_d = nc.dram_tensor("o", (C, W), f16, kind="ExternalOutput").ap()

    with tile.TileContext(nc) as tc, ExitStack() as ctx:
        consts = ctx.enter_context(tc.tile_pool(name="consts", bufs=1))
        sm_pool = ctx.enter_context(tc.tile_pool(name="sm", bufs=2))
        at_psum = ctx.enter_context(tc.tile_pool(name="at", bufs=2, space="PSUM"))
        g_psum = ctx.enter_context(tc.tile_pool(name="g", bufs=2, space="PSUM"))
        out_psum = ctx.enter_context(tc.tile_pool(name="out", bufs=2, space="PSUM"))
        s_psum = ctx.enter_context(tc.tile_pool(name="sp", bufs=1, space="PSUM"))

        qT_t = consts.tile([C, W], f16)
        kT_t = consts.tile([C, W], f16)
        v_t = consts.tile([C, VW], f16)
        nc.sync.dma_start(qT_t, qT_d)
        nc.sync.dma_start(kT_t, kT_d)
        nc.sync.dma_start(v_t, v_d)

        # one-time consts: causal tri mask + qfb off-pair zero blocks
        tri = consts.tile([C, C], f16)
        nc.gpsimd.memset(tri, 0.0)
        nc.gpsimd.affine_select(
            out=tri, in_=tri, compare_op=OP.is_gt, fill=1.0,
            base=0, pattern=[[-1, C]], channel_multiplier=1,
        )
        qfb = consts.tile([C, PAIRS * W], f16)
        nc.gpsimd.memset(qfb[D:C, 0:W], 0.0)
        nc.gpsimd.memset(qfb[0:D, W:2 * W], 0.0)

        # feature maps: f = min(exp(x), max(x+1, 1))
        eq = consts.tile([C, W], f16)
        tq = consts.tile([C, W], f16)
        qf = consts.tile([C, W], f16)
        nc.scalar.activation(eq, qT_t, AF.Exp)
        nc.vector.tensor_scalar(out=tq, in0=qT_t, scalar1=1.0, scalar2=1.0,
                                op0=OP.add, op1=OP.max)
        nc.vector.tensor_tensor(out=qf, in0=eq, in1=tq, op=OP.min)

        ek = consts.tile([C, W], f16)
        tk = consts.tile([C, W], f16)
        kfT = consts.tile([C, W], f16)
        nc.scalar.activation(ek, kT_t, AF.Exp)
        nc.vector.tensor_scalar(out=tk, in0=kT_t, scalar1=1.0, scalar2=1.0,
                                op0=OP.add, op1=OP.max)
        nc.vector.tensor_tensor(out=kfT, in0=ek, in1=tk, op=OP.min)

        # natural-layout kf[i, (c, pd)] via SBUF->SBUF xbar transpose
        kf = consts.tile([C, W], f16)
        kf3 = kf.rearrange("i (c pd) -> i c pd", c=NCH)
        nc.sync.dma_start(kf3, kfT, transpose=True)

        # blocked qfb via partition-sliced SBUF->SBUF copies of dense qf
        nc.sync.dma_start(qfb[0:D, 0:W], qf[0:D, :])
        nc.sync.dma_start(qfb[D:C, W:2 * W], qf[D:C, :])
        qfb4 = qfb.rearrange("r (p x) -> r p x", p=PAIRS)

        # running state S in PSUM (accumulated by PE across all chunks)
        S_full = s_psum.tile([C, 512], f32)
        S_ps = S_full[:, 0:SW]

        stage = consts.tile([C, W], f16)
        recip_sb = consts.tile([C, 2 * NCH], f32)

        tri_b = bass.AP(
            tensor=tri.tensor, offset=tri.offset,
            ap=[list(tri.ap[0]), [0, 4], [1, C]],
        )

        for s in range(NCH // 2):
            c0, c1 = 2 * s, 2 * s + 1
            t0 = slice(c0 * C, (c0 + 1) * C)
            t1 = slice(c1 * C, (c1 + 1) * C)

            # snapshot S (state through chunk c0-1) BEFORE this step's updates
            if s > 0:
                S_sb = sm_pool.tile([C, SW], f16, tag="s_sb")
                nc.scalar.copy(S_sb, S_ps)

            # AT for both chunks, both pairs: one PSUM bank [128, 512]
            at_ps = at_psum.tile([C, 4 * C], f32, tag="at")
            nc.tensor.matmul(at_ps[:, 0:2 * C], kfT[:, t0], qfb4[:, :, t0],
                             start=True, stop=False, skip_group_check=True)
            nc.tensor.matmul(at_ps[:, 2 * C:4 * C], kfT[:, t1], qfb4[:, :, t1],
                             start=False, stop=True, skip_group_check=True)

            # G_c0 = Kf_c0^T Vaug_c0 (cross source) + S updates (same stationary)
            g_full = g_psum.tile([C, 512], f32, tag="g")
            g_ps = g_full[:, 0:SW]
            nc.tensor.matmul(g_ps, kf[:, t0], v_t[:, c0 * SW:(c0 + 1) * SW],
                             start=True, stop=True, skip_group_check=True)
            nc.tensor.matmul(S_ps, kf[:, t0], v_t[:, c0 * SW:(c0 + 1) * SW],
                             start=(s == 0), stop=False, skip_group_check=True)
            nc.tensor.matmul(S_ps, kf[:, t1], v_t[:, c1 * SW:(c1 + 1) * SW],
                             start=False, stop=(s == NCH // 2 - 1),
                             skip_group_check=True)
            G_sb = sm_pool.tile([C, SW], f16, tag="g_sb")
            nc.scalar.copy(G_sb, g_ps)

            # mask all 4 tri blocks in one DVE op (broadcast tri)
            atm = sm_pool.tile([C, 4 * C], f16, tag="atm")
            at3 = at_ps.rearrange("i (b x) -> i b x", b=4)
            atm3 = atm.rearrange("i (b x) -> i b x", b=4)
            nc.vector.tensor_tensor(out=atm3, in0=at3, in1=tri_b, op=OP.mult)

            # out accumulation [c0p0 | c0p1 | c1p0 | c1p1] (65 cols each)
            out_full = out_psum.tile([C, 512], f32, tag="out")
            out_ps = out_full[:, 0:4 * DV]
            first = [True]

            def omm(lhsT, rhs, blk, stop=False):
                nc.tensor.matmul(out_ps[:, blk * DV:(blk + 1) * DV], lhsT, rhs,
                                 start=first[0], stop=stop,
                                 skip_group_check=True)
                first[0] = False

            for p in range(PAIRS):
                vs = slice(p * DV, (p + 1) * DV)
                if s > 0:
                    omm(qfb[:, p * W + t0.start:p * W + t0.stop], S_sb[:, vs], p)
                    omm(qfb[:, p * W + t1.start:p * W + t1.stop], S_sb[:, vs],
                        2 + p)
                omm(qfb[:, p * W + t1.start:p * W + t1.stop], G_sb[:, vs], 2 + p)
            for p in range(PAIRS):
                nv0 = slice(c0 * SW + p * DV, c0 * SW + (p + 1) * DV)
                nv1 = slice(c1 * SW + p * DV, c1 * SW + (p + 1) * DV)
                omm(atm[:, p * C:(p + 1) * C], v_t[:, nv0], p)
                omm(atm[:, (2 + p) * C:(3 + p) * C], v_t[:, nv1], 2 + p,
                    stop=(p == PAIRS - 1))

            # denominators -> reciprocals (collected); numerators -> stage
            ob = out_ps.rearrange("i (b x) -> i b x", b=4)
            den = ob[:, :, D:D + 1]
            rout = recip_sb[:, 4 * s:4 * (s + 1)].rearrange(
                "i (b x) -> i b x", x=1)
            nc.vector.reciprocal(rout, den)
            num = out_ps.rearrange(
                "i (c p dv) -> i c p dv", c=2, p=PAIRS)[:, :, :, 0:D]
            st4 = stage.rearrange(
                "i (c p d) -> i c p d", c=NCH, p=PAIRS)[:, c0:c0 + 2]
            nc.scalar.activation(st4, num, AF.Copy)

            # deferred normalization + output DMA every 2 steps
            if s % 2 == 1:
                g4 = slice((s - 1) * 2 * C, (s + 1) * 2 * C)
                stg = stage[:, g4].rearrange(
                    "i (c p d) -> i c p d", c=4, p=PAIRS)
                rsl = recip_sb[:, 4 * (s - 1):4 * (s + 1)]
                rec_b = bass.AP(
                    tensor=rsl.tensor, offset=rsl.offset,
                    ap=[list(rsl.ap[0]), [2, 4], [1, 2], [0, D]],
                )
                nc.vector.tensor_tensor(out=stg, in0=stg, in1=rec_b, op=OP.mult)
                nc.sync.dma_start(o_d[:, g4], stage[:, g4])

    nc.compile()
    return nc


_nc_cache = None


def _get_nc():
    global _nc_cache
    if _nc_cache is None:
        _nc_cache = build_kernel()
    return _nc_cache


def _core_pairs(x, core):
    flat = x.transpose(0, 2, 1, 3).reshape(N * H, L, D)
    return flat[2 * core:2 * core + 2]          # (2, L, D) fp32


def _t_layout(xc):
    # (2, L, D) -> (128, 2048) [(64p + d), (128c + i)]
    return np.ascontiguousarray(
        xc.reshape(PAIRS, NCH, C, D).transpose(0, 3, 1, 2).reshape(C, W)
    ).astype(np.float16)


def _v_layout(xc):
    # (2, L, D) -> (128, 2080) [i, (c, p, dv)] with ones at dv=64
    v4 = xc.reshape(PAIRS, NCH, C, D).transpose(2, 1, 0, 3)  # (C, NCH, PAIRS, D)
    vaug = np.concatenate(
        [v4, np.ones((C, NCH, PAIRS, 1), v4.dtype)], axis=-1)
    return np.ascontiguousarray(vaug.reshape(C, VW)).astype(np.float16)


def make_in_maps(queries, keys, values):
    in_maps = []
    for core in range(8):
        in_maps.append({
            "qT": _t_layout(_core_pairs(queries, core)),
            "kT": _t_layout(_core_pairs(keys, core)),
            "v": _v_layout(_core_pairs(values, core)),
        })
    return in_maps


def _unpack_out(o):
    # (128, 2048) [i', (c, p, d)] -> (PAIRS, L, D)
    return o.reshape(C, NCH, PAIRS, D).transpose(2, 1, 0, 3).reshape(PAIRS, L, D)


def kernel(queries, keys, values):
    nc = _get_nc()
    in_maps = make_in_maps(queries, keys, values)
    res = run_bass_kernel_spmd(nc, in_maps, core_ids=list(range(8)))
    out = np.zeros((N, L, H, D), np.float32)
    for core in range(8):
        oc = _unpack_out(res.results[core]["o"].astype(np.float32))
        for p in range(PAIRS):
            flat = 2 * core + p
            out[flat // H, :, flat % H, :] = oc[p]
    return out


# revision 5
# speedup vs baseline: 1.1014x; 1.0290x over previous
"""Causal linear attention (elu+1 feature map) on 8 Trainium2 NeuronCores.

Full inputs (n=2, l=2048, h=8, d=64) fp32 are sharded over the 16 (n,h)
head-sequences: core i handles pairs (2i, 2i+1). Single-pass design (all 16
chunks of C=128 in one group), stride-2 scan (2 chunks per step).

Per step (chunks c0=2s, c1=2s+1):
  AT(c0), AT(c1): one matmul each (stationary kfT chunk, moving blocked qfb
  both pairs) into one PSUM bank [128, 512] = [ATc0p0|ATc0p1|ATc1p0|ATc1p1].
  Causal masking = ONE DVE multiply with a broadcast [128,128] tri mask
  (stride-0 AP over the 4 blocks).

  Cross-chunk term (c0 -> c1) never materializes a 128x128 AT block: the
  rank-64 factorization G_c0 = Kf_c0^T @ Vaug_c0 (the chunk's state
  increment, one extra matmul into its own PSUM bank + one 130-col ACT copy)
  gives cross = Qf_c1 @ G_c0 via the same stationary as Q@S.

  out_ps [128, 260] accumulates Q@S_snap + Q@G + ATm^T V per (chunk, pair),
  with a trailing denominator column from vaug's baked-in ones column.
  First matmul into each PSUM bank uses start=True (clears the bank; no
  zeros-init matmuls).

  Normalization is deferred: numerators are evacuated PSUM->SBUF stage by
  ScalarE (Copy), reciprocals collected per step into a [128, 32] tile by
  DVE, and every 2 steps one in-place f16 DVE multiply (recip broadcast via
  stride-0 AP) normalizes 512 staged columns, followed by the output DMA.

  The feature map f = min(exp(x), max(x+1, 1)) = elu(x)+1 runs once on qT
  and once on kT (exp on ScalarE, the rest on DVE); the natural-layout kf
  needed as the G/S stationary comes from an SBUF->SBUF DMA xbar transpose
  of the *feature-mapped* kfT (3D out AP [i, c, pd]), and the blocked qfb
  (off-pair blocks zero, for the shared-stationary AT trick) is assembled
  by two partition-sliced SBUF->SBUF DMAs from the dense qf plus one-time
  GpSimd memsets of the off-blocks.

Host layouts (fp16, all DMAs contiguous):
  qT, kT: (128, 2048)  [(64p + d), (128c + i)]   (host-transposed)
  v     : (128, 2080)  [i, (c, p, dv)] dv=65, ones baked in at dv=64
  o     : (128, 2048)  [i', (c, p, d)] fp16
"""
import numpy as np
from contextlib import ExitStack

import concourse.bacc as bacc
import concourse.bass as bass
import concourse.tile as tile
from concourse import mybir
from concourse.bass_utils import run_bass_kernel_spmd

N, L, H, D = 2, 2048, 8, 64
C = 128                 # chunk length
NCH = L // C            # 16 chunks
PAIRS = 2
W = NCH * PAIRS * D     # 2048
DV = D + 1              # 65: value cols + denominator ones col
VW = NCH * PAIRS * DV   # 2080
SW = PAIRS * DV         # 130

f16 = mybir.dt.float16
f32 = mybir.dt.float32
AF = mybir.ActivationFunctionType
OP = mybir.AluOpType


def build_kernel():
    nc = bacc.Bacc("TRN2", target_bir_lowering=False, debug=False, num_devices=8)
    qT_d = nc.dram_tensor("qT", (C, W), f16, kind="ExternalInput").ap()
    kT_d = nc.dram_tensor("kT", (C, W), f16, kind="ExternalInput").ap()
    v_d = nc.dram_tensor("v", (C, VW), f16, kind="ExternalInput").ap()
    o_d = nc.dram_tensor("o", (C, W), f16, kind="ExternalOutput").ap()

    with tile.TileContext(nc) as tc, ExitStack() as ctx:
        consts = ctx.enter_context(tc.tile_pool(name="consts", bufs=1))
        sm_pool = ctx.enter_context(tc.tile_pool(name="sm", bufs=2))
        atm_pool = ctx.enter_context(tc.tile_pool(name="atm", bufs=3))
        at_psum = ctx.enter_context(tc.tile_pool(name="at", bufs=3, space="PSUM"))
        g_psum = ctx.enter_context(tc.tile_pool(name="g", bufs=2, space="PSUM"))
        out_psum = ctx.enter_context(tc.tile_pool(name="out", bufs=2, space="PSUM"))
        s_psum = ctx.enter_context(tc.tile_pool(name="sp", bufs=1, space="PSUM"))

        qT_t = consts.tile([C, W], f16)
        kT_t = consts.tile([C, W], f16)
        v_t = consts.tile([C, VW], f16)
        nc.sync.dma_start(kT_t, kT_d)
        nc.sync.dma_start(qT_t, qT_d)
        nc.sync.dma_start(v_t, v_d)

        # one-time consts: causal tri mask + qfb off-pair zero blocks
        tri = consts.tile([C, C], f16)
        nc.gpsimd.memset(tri, 0.0)
        nc.gpsimd.affine_select(
            out=tri, in_=tri, compare_op=OP.is_gt, fill=1.0,
            base=0, pattern=[[-1, C]], channel_multiplier=1,
        )
        qfb = consts.tile([C, PAIRS * W], f16)
        nc.gpsimd.memset(qfb[D:C, 0:W], 0.0)
        nc.gpsimd.memset(qfb[0:D, W:2 * W], 0.0)

        # feature maps f = min(exp(x), max(x+1, 1)), split in halves to
        # pipeline ACT (exp) / DVE (t, min) / DMA (transpose, scatter)
        eq = consts.tile([C, W], f16)
        tq = consts.tile([C, W], f16)
        qf = consts.tile([C, W], f16)
        ek = consts.tile([C, W], f16)
        tk = consts.tile([C, W], f16)
        kfT = consts.tile([C, W], f16)
        kf = consts.tile([C, W], f16)
        kf3 = kf.rearrange("i (c pd) -> i c pd", c=NCH)
        HW_ = W // 2
        for h in range(2):
            hs = slice(h * HW_, (h + 1) * HW_)
            nc.scalar.activation(ek[:, hs], kT_t[:, hs], AF.Exp)
            nc.vector.tensor_scalar(out=tk[:, hs], in0=kT_t[:, hs],
                                    scalar1=1.0, scalar2=1.0,
                                    op0=OP.add, op1=OP.max)
            nc.vector.tensor_tensor(out=kfT[:, hs], in0=ek[:, hs],
                                    in1=tk[:, hs], op=OP.min)
            # natural-layout kf[i, (c, pd)] via SBUF->SBUF xbar transpose
            nc.sync.dma_start(kf3[:, h * NCH // 2:(h + 1) * NCH // 2],
                              kfT[:, hs], transpose=True)
        for h in range(2):
            hs = slice(h * HW_, (h + 1) * HW_)
            nc.scalar.activation(eq[:, hs], qT_t[:, hs], AF.Exp)
            nc.vector.tensor_scalar(out=tq[:, hs], in0=qT_t[:, hs],
                                    scalar1=1.0, scalar2=1.0,
                                    op0=OP.add, op1=OP.max)
            nc.vector.tensor_tensor(out=qf[:, hs], in0=eq[:, hs],
                                    in1=tq[:, hs], op=OP.min)
            # blocked qfb via partition-sliced SBUF->SBUF copies of dense qf
            nc.sync.dma_start(qfb[0:D, hs], qf[0:D, hs])
            nc.sync.dma_start(
                qfb[D:C, W + h * HW_:W + (h + 1) * HW_], qf[D:C, hs])
        qfb4 = qfb.rearrange("r (p x) -> r p x", p=PAIRS)

        # running state S in PSUM (accumulated by PE across all chunks)
        S_full = s_psum.tile([C, 512], f32)
        S_ps = S_full[:, 0:SW]

        stage = consts.tile([C, W], f16)
        recip_sb = consts.tile([C, 2 * NCH], f32)

        tri_b = bass.AP(
            tensor=tri.tensor, offset=tri.offset,
            ap=[list(tri.ap[0]), [0, 4], [1, C]],
        )

        for s in range(NCH // 2):
            c0, c1 = 2 * s, 2 * s + 1
            t0 = slice(c0 * C, (c0 + 1) * C)
            t1 = slice(c1 * C, (c1 + 1) * C)

            # snapshot S (state through chunk c0-1) BEFORE this step's updates
            if s > 0:
                S_sb = sm_pool.tile([C, SW], f16, tag="s_sb")
                nc.scalar.copy(S_sb, S_ps)

            # AT for both chunks, both pairs: one PSUM bank [128, 512]
            at_ps = at_psum.tile([C, 4 * C], f32, tag="at")
            nc.tensor.matmul(at_ps[:, 0:2 * C], kfT[:, t0], qfb4[:, :, t0],
                             start=True, stop=False, skip_group_check=True)
            nc.tensor.matmul(at_ps[:, 2 * C:4 * C], kfT[:, t1], qfb4[:, :, t1],
                             start=False, stop=True, skip_group_check=True)

            # G_c0 = Kf_c0^T Vaug_c0 (cross source) + S updates (same stationary)
            g_full = g_psum.tile([C, 512], f32, tag="g")
            g_ps = g_full[:, 0:SW]
            nc.tensor.matmul(g_ps, kf[:, t0], v_t[:, c0 * SW:(c0 + 1) * SW],
                             start=True, stop=True, skip_group_check=True)
            nc.tensor.matmul(S_ps, kf[:, t0], v_t[:, c0 * SW:(c0 + 1) * SW],
                             start=(s == 0), stop=False, skip_group_check=True)
            nc.tensor.matmul(S_ps, kf[:, t1], v_t[:, c1 * SW:(c1 + 1) * SW],
                             start=False, stop=(s == NCH // 2 - 1),
                             skip_group_check=True)
            G_sb = sm_pool.tile([C, SW], f16, tag="g_sb")
            nc.scalar.copy(G_sb, g_ps)

            # mask all 4 tri blocks in one DVE op (broadcast tri)
            atm = atm_pool.tile([C, 4 * C], f16, tag="atm")
            at3 = at_ps.rearrange("i (b x) -> i b x", b=4)
            atm3 = atm.rearrange("i (b x) -> i b x", b=4)
            nc.vector.tensor_tensor(out=atm3, in0=at3, in1=tri_b, op=OP.mult)

            # out accumulation [c0p0 | c0p1 | c1p0 | c1p1] (65 cols each)
            out_full = out_psum.tile([C, 512], f32, tag="out")
            out_ps = out_full[:, 0:4 * DV]
            first = [True]

            def omm(lhsT, rhs, blk, stop=False):
                nc.tensor.matmul(out_ps[:, blk * DV:(blk + 1) * DV], lhsT, rhs,
                                 start=first[0], stop=stop,
                                 skip_group_check=True)
                first[0] = False

            for p in range(PAIRS):
                vs = slice(p * DV, (p + 1) * DV)
                if s > 0:
                    omm(qfb[:, p * W + t0.start:p * W + t0.stop], S_sb[:, vs], p)
                    omm(qfb[:, p * W + t1.start:p * W + t1.stop], S_sb[:, vs],
                        2 + p)
                omm(qfb[:, p * W + t1.start:p * W + t1.stop], G_sb[:, vs], 2 + p)
            for p in range(PAIRS):
                nv0 = slice(c0 * SW + p * DV, c0 * SW + (p + 1) * DV)
                nv1 = slice(c1 * SW + p * DV, c1 * SW + (p + 1) * DV)
                omm(atm[:, p * C:(p + 1) * C], v_t[:, nv0], p)
                omm(atm[:, (2 + p) * C:(3 + p) * C], v_t[:, nv1], 2 + p,
                    stop=(p == PAIRS - 1))

            # denominators -> reciprocals (collected); numerators -> stage
            ob = out_ps.rearrange("i (b x) -> i b x", b=4)
            den = ob[:, :, D:D + 1]
            rout = recip_sb[:, 4 * s:4 * (s + 1)].rearrange(
                "i (b x) -> i b x", x=1)
            nc.vector.reciprocal(rout, den)
            num = out_ps.rearrange(
                "i (c p dv) -> i c p dv", c=2, p=PAIRS)[:, :, :, 0:D]
            st4 = stage.rearrange(
                "i (c p d) -> i c p d", c=NCH, p=PAIRS)[:, c0:c0 + 2]
            nc.scalar.activation(st4, num, AF.Copy)

            # deferred normalization + output DMA per step
            g4 = slice(s * 2 * C, (s + 1) * 2 * C)
            stg = stage[:, g4].rearrange(
                "i (c p d) -> i c p d", c=2, p=PAIRS)
            rsl = recip_sb[:, 4 * s:4 * (s + 1)]
            rec_b = bass.AP(
                tensor=rsl.tensor, offset=rsl.offset,
                ap=[list(rsl.ap[0]), [2, 2], [1, 2], [0, D]],
            )
            nc.vector.tensor_tensor(out=stg, in0=stg, in1=rec_b, op=OP.mult)
            nc.sync.dma_start(o_d[:, g4], stage[:, g4])

    nc.compile()
    return nc


_nc_cache = None


def _get_nc():
    global _nc_cache
    if _nc_cache is None:
        _nc_cache = build_kernel()
    return _nc_cache


def _core_pairs(x, core):
    flat = x.transpose(0, 2, 1, 3).reshape(N * H, L, D)
    return flat[2 * core:2 * core + 2]          # (2, L, D) fp32


def _t_layout(xc):
    # (2, L, D) -> (128, 2048) [(64p + d), (128c + i)]
    return np.ascontiguousarray(
        xc.reshape(PAIRS, NCH, C, D).transpose(0, 3, 1, 2).reshape(C, W)
    ).astype(np.float16)


def _v_layout(xc):
    # (2, L, D) -> (128, 2080) [i, (c, p, dv)] with ones at dv=64
    v4 = xc.reshape(PAIRS, NCH, C, D).transpose(2, 1, 0, 3)  # (C, NCH, PAIRS, D)
    vaug = np.concatenate(
        [v4, np.ones((C, NCH, PAIRS, 1), v4.dtype)], axis=-1)
    return np.ascontiguousarray(vaug.reshape(C, VW)).astype(np.float16)


def make_in_maps(queries, keys, values):
    in_maps = []
    for core in range(8):
        in_maps.append({
            "qT": _t_layout(_core_pairs(queries, core)),
            "kT": _t_layout(_core_pairs(keys, core)),
            "v": _v_layout(_core_pairs(values, core)),
        })
    return in_maps


def _unpack_out(o):
    # (128, 2048) [i', (c, p, d)] -> (PAIRS, L, D)
    return o.reshape(C, NCH, PAIRS, D).transpose(2, 1, 0, 3).reshape(PAIRS, L, D)


def kernel(queries, keys, values):
    nc = _get_nc()
    in_maps = make_in_maps(queries, keys, values)
    res = run_bass_kernel_spmd(nc, in_maps, core_ids=list(range(8)))
    out = np.zeros((N, L, H, D), np.float32)
    for core in range(8):
        oc = _unpack_out(res.results[core]["o"].astype(np.float32))
        for p in range(PAIRS):
            flat = 2 * core + p
            out[flat // H, :, flat % H, :] = oc[p]
    return out


# revision 7
# speedup vs baseline: 1.1341x; 1.0297x over previous
"""Causal linear attention (elu+1 feature map) on 8 Trainium2 NeuronCores.

Full inputs (n=2, l=2048, h=8, d=64) fp32 are sharded over the 16 (n,h)
head-sequences: core i handles pairs (2i, 2i+1). Single-pass design (all 16
chunks of C=128 in one group), stride-2 scan (2 chunks per step).

Per step (chunks c0=2s, c1=2s+1):
  AT(c0), AT(c1): one matmul each (stationary kfT chunk, moving blocked qfb
  both pairs) into one PSUM bank [128, 512] = [ATc0p0|ATc0p1|ATc1p0|ATc1p1].
  Causal masking = ONE DVE multiply with a broadcast [128,128] tri mask
  (stride-0 AP over the 4 blocks).

  Cross-chunk term (c0 -> c1) never materializes a 128x128 AT block: the
  rank-64 factorization G_c0 = Kf_c0^T @ Vaug_c0 (the chunk's state
  increment, one extra matmul into its own PSUM bank + one 130-col ACT copy)
  gives cross = Qf_c1 @ G_c0 via the same stationary as Q@S.

  out_ps [128, 260] accumulates Q@S_snap + Q@G + ATm^T V per (chunk, pair),
  with a trailing denominator column from vaug's baked-in ones column.
  First matmul into each PSUM bank uses start=True (clears the bank; no
  zeros-init matmuls).

  Normalization is deferred: numerators are evacuated PSUM->SBUF stage by
  ScalarE (Copy), reciprocals collected per step into a [128, 32] tile by
  DVE, and every 2 steps one in-place f16 DVE multiply (recip broadcast via
  stride-0 AP) normalizes 512 staged columns, followed by the output DMA.

  The feature map f = min(exp(x), max(x+1, 1)) = elu(x)+1 runs once on qT
  and once on kT (exp on ScalarE, the rest on DVE); the natural-layout kf
  needed as the G/S stationary comes from an SBUF->SBUF DMA xbar transpose
  of the *feature-mapped* kfT (3D out AP [i, c, pd]), and the blocked qfb
  (off-pair blocks zero, for the shared-stationary AT trick) is assembled
  by two partition-sliced SBUF->SBUF DMAs from the dense qf plus one-time
  GpSimd memsets of the off-blocks.

Host layouts (fp16, all DMAs contiguous):
  qT, kT: (128, 2048)  [(64p + d), (128c + i)]   (host-transposed)
  v     : (128, 2080)  [i, (c, p, dv)] dv=65, ones baked in at dv=64
  o     : (128, 2048)  [i', (c, p, d)] fp16
"""
import numpy as np
from contextlib import ExitStack

import concourse.bacc as bacc
import concourse.bass as bass
import concourse.tile as tile
from concourse import mybir
from concourse.bass_utils import run_bass_kernel_spmd

N, L, H, D = 2, 2048, 8, 64
C = 128                 # chunk length
NCH = L // C            # 16 chunks
PAIRS = 2
W = NCH * PAIRS * D     # 2048
DV = D + 1              # 65: value cols + denominator ones col
VW = NCH * PAIRS * DV   # 2080
SW = PAIRS * DV         # 130

f16 = mybir.dt.float16
f32 = mybir.dt.float32
AF = mybir.ActivationFunctionType
OP = mybir.AluOpType


def build_kernel():
    nc = bacc.Bacc("TRN2", target_bir_lowering=False, debug=False, num_devices=8)
    qT_d = nc.dram_tensor("qT", (C, W), f16, kind="ExternalInput").ap()
    kT_d = nc.dram_tensor("kT", (C, W), f16, kind="ExternalInput").ap()
    v_d = nc.dram_tensor("v", (C, VW), f16, kind="ExternalInput").ap()
    o_d = nc.dram_tensor("o", (C, W), f16, kind="ExternalOutput").ap()

    with tile.TileContext(nc) as tc, ExitStack() as ctx:
        consts = ctx.enter_context(tc.tile_pool(name="consts", bufs=1))
        sm_pool = ctx.enter_context(tc.tile_pool(name="sm", bufs=2))
        atm_pool = ctx.enter_context(tc.tile_pool(name="atm", bufs=3))
        at_psum = ctx.enter_context(tc.tile_pool(name="at", bufs=3, space="PSUM"))
        out_psum = ctx.enter_context(tc.tile_pool(name="out", bufs=2, space="PSUM"))
        s_psum = ctx.enter_context(tc.tile_pool(name="sp", bufs=1, space="PSUM"))

        qT_t = consts.tile([C, W], f16)
        kT_t = consts.tile([C, W], f16)
        v_t = consts.tile([C, VW], f16)
        HW_ = W // 2
        nc.sync.dma_start(kT_t[:, 0:HW_], kT_d[:, 0:HW_])
        nc.sync.dma_start(kT_t[:, HW_:W], kT_d[:, HW_:W])
        nc.sync.dma_start(v_t, v_d)
        nc.sync.dma_start(qT_t[:, 0:HW_], qT_d[:, 0:HW_])
        nc.sync.dma_start(qT_t[:, HW_:W], qT_d[:, HW_:W])

        # one-time consts: causal tri mask + qfb off-pair zero blocks
        tri = consts.tile([C, C], f16)
        nc.gpsimd.memset(tri, 0.0)
        nc.gpsimd.affine_select(
            out=tri, in_=tri, compare_op=OP.is_gt, fill=1.0,
            base=0, pattern=[[-1, C]], channel_multiplier=1,
        )
        qfb = consts.tile([C, PAIRS * W], f16)
        nc.gpsimd.memset(qfb[D:C, 0:W], 0.0)
        nc.gpsimd.memset(qfb[0:D, W:2 * W], 0.0)

        # feature maps f = min(exp(x), max(x+1, 1)), split in halves to
        # pipeline ACT (exp) / DVE (t, min) / DMA (transpose)
        eq = consts.tile([C, W], f16)
        tq = consts.tile([C, W], f16)
        ek = consts.tile([C, W], f16)
        tk = consts.tile([C, W], f16)
        kfT = consts.tile([C, W], f16)
        kf = consts.tile([C, W], f16)
        kf3 = kf.rearrange("i (c pd) -> i c pd", c=NCH)
        for h in range(2):
            hs = slice(h * HW_, (h + 1) * HW_)
            nc.scalar.activation(ek[:, hs], kT_t[:, hs], AF.Exp)
            nc.vector.tensor_scalar(out=tk[:, hs], in0=kT_t[:, hs],
                                    scalar1=1.0, scalar2=1.0,
                                    op0=OP.add, op1=OP.max)
            nc.vector.tensor_tensor(out=kfT[:, hs], in0=ek[:, hs],
                                    in1=tk[:, hs], op=OP.min)
            # natural-layout kf[i, (c, pd)] via SBUF->SBUF xbar transpose
            nc.sync.dma_start(kf3[:, h * NCH // 2:(h + 1) * NCH // 2],
                              kfT[:, hs], transpose=True)

        def fmap_q_half(h):
            hs = slice(h * HW_, (h + 1) * HW_)
            nc.scalar.activation(eq[:, hs], qT_t[:, hs], AF.Exp)
            nc.vector.tensor_scalar(out=tq[:, hs], in0=qT_t[:, hs],
                                    scalar1=1.0, scalar2=1.0,
                                    op0=OP.add, op1=OP.max)
            # blocked qfb written directly (off-pair blocks stay memset-zero)
            nc.vector.tensor_tensor(out=qfb[0:D, hs], in0=eq[0:D, hs],
                                    in1=tq[0:D, hs], op=OP.min)
            nc.vector.tensor_tensor(
                out=qfb[D:C, W + h * HW_:W + (h + 1) * HW_],
                in0=eq[D:C, hs], in1=tq[D:C, hs], op=OP.min)

        fmap_q_half(0)
        qfb4 = qfb.rearrange("r (p x) -> r p x", p=PAIRS)

        # running state S + per-step G in one 3-bank PSUM region:
        # bank 0 = S, banks 1/2 = G (rotating) -- lets one ACT copy per step
        # snapshot both S and G with a single strided read
        SG_full = s_psum.tile([C, 3 * 512], f32)
        S_ps = SG_full[:, 0:SW]

        stage = consts.tile([C, W], f16)
        recip_sb = consts.tile([C, 2 * NCH], f32)

        tri_b = bass.AP(
            tensor=tri.tensor, offset=tri.offset,
            ap=[list(tri.ap[0]), [0, 4], [1, C]],
        )

        for s in range(NCH // 2):
            c0, c1 = 2 * s, 2 * s + 1
            t0 = slice(c0 * C, (c0 + 1) * C)
            t1 = slice(c1 * C, (c1 + 1) * C)

            # AT for both chunks, both pairs: one PSUM bank [128, 512]
            at_ps = at_psum.tile([C, 4 * C], f32, tag="at")
            nc.tensor.matmul(at_ps[:, 0:2 * C], kfT[:, t0], qfb4[:, :, t0],
                             start=True, stop=False, skip_group_check=True)
            nc.tensor.matmul(at_ps[:, 2 * C:4 * C], kfT[:, t1], qfb4[:, :, t1],
                             start=False, stop=True, skip_group_check=True)

            # G_c0 = Kf_c0^T Vaug_c0 (cross source; also the S increment)
            goff = 512 * (1 + s % 2)
            g_ps = SG_full[:, goff:goff + SW]
            nc.tensor.matmul(g_ps, kf[:, t0], v_t[:, c0 * SW:(c0 + 1) * SW],
                             start=True, stop=True, skip_group_check=True)
            # one ACT copy snapshots S (pre-update) and G together
            SG_sb = sm_pool.tile([C, 2 * SW], f16, tag="sg_sb")
            S_sb = SG_sb[:, 0:SW]
            G_sb = SG_sb[:, SW:2 * SW]
            if s == 0:
                nc.scalar.copy(G_sb, g_ps)   # S not yet written
            else:
                sg_src = bass.AP(
                    tensor=SG_full.tensor, offset=SG_full.offset,
                    ap=[list(SG_full.ap[0]), [goff, 2], [1, SW]],
                )
                sg_dst = SG_sb.rearrange("i (b x) -> i b x", b=2)
                nc.scalar.activation(sg_dst, sg_src, AF.Copy)

            # mask all 4 tri blocks in one DVE op (broadcast tri)
            atm = atm_pool.tile([C, 4 * C], f16, tag="atm")
            at3 = at_ps.rearrange("i (b x) -> i b x", b=4)
            atm3 = atm.rearrange("i (b x) -> i b x", b=4)
            nc.vector.tensor_tensor(out=atm3, in0=at3, in1=tri_b, op=OP.mult)

            # out accumulation [c0p0 | c0p1 | c1p0 | c1p1] (65 cols each)
            out_full = out_psum.tile([C, 512], f32, tag="out")
            out_ps = out_full[:, 0:4 * DV]
            first = [True]

            def omm(lhsT, rhs, blk, stop=False):
                nc.tensor.matmul(out_ps[:, blk * DV:(blk + 1) * DV], lhsT, rhs,
                                 start=first[0], stop=stop,
                                 skip_group_check=True)
                first[0] = False

            for p in range(PAIRS):
                vs = slice(p * DV, (p + 1) * DV)
                if s > 0:
                    omm(qfb[:, p * W + t0.start:p * W + t0.stop], S_sb[:, vs], p)
                    omm(qfb[:, p * W + t1.start:p * W + t1.stop], S_sb[:, vs],
                        2 + p)
                omm(qfb[:, p * W + t1.start:p * W + t1.stop], G_sb[:, vs], 2 + p)
            for p in range(PAIRS):
                nv0 = slice(c0 * SW + p * DV, c0 * SW + (p + 1) * DV)
                nv1 = slice(c1 * SW + p * DV, c1 * SW + (p + 1) * DV)
                omm(atm[:, p * C:(p + 1) * C], v_t[:, nv0], p)
                omm(atm[:, (2 + p) * C:(3 + p) * C], v_t[:, nv1], 2 + p,
                    stop=(p == PAIRS - 1))

            # S updates AFTER the out-MMs: they only must precede the NEXT
            # step's combined S|G snapshot (keeps PE from stalling on it)
            nc.tensor.matmul(S_ps, kf[:, t0], v_t[:, c0 * SW:(c0 + 1) * SW],
                             start=(s == 0), stop=False, skip_group_check=True)
            nc.tensor.matmul(S_ps, kf[:, t1], v_t[:, c1 * SW:(c1 + 1) * SW],
                             start=False, stop=(s == NCH // 2 - 1),
                             skip_group_check=True)

            if s == 1:
                fmap_q_half(1)

            # denominators -> reciprocals (collected); numerators -> stage
            ob = out_ps.rearrange("i (b x) -> i b x", b=4)
            den = ob[:, :, D:D + 1]
            rout = recip_sb[:, 4 * s:4 * (s + 1)].rearrange(
                "i (b x) -> i b x", x=1)
            nc.vector.reciprocal(rout, den)
            num = out_ps.rearrange(
                "i (c p dv) -> i c p dv", c=2, p=PAIRS)[:, :, :, 0:D]
            st4 = stage.rearrange(
                "i (c p d) -> i c p d", c=NCH, p=PAIRS)[:, c0:c0 + 2]
            nc.scalar.activation(st4, num, AF.Copy)

            # deferred normalization + output DMA per step
            g4 = slice(s * 2 * C, (s + 1) * 2 * C)
            stg = stage[:, g4].rearrange(
                "i (c p d) -> i c p d", c=2, p=PAIRS)
            rsl = recip_sb[:, 4 * s:4 * (s + 1)]
            rec_b = bass.AP(
                tensor=rsl.tensor, offset=rsl.offset,
                ap=[list(rsl.ap[0]), [2, 2], [1, 2], [0, D]],
            )
            nc.vector.tensor_tensor(out=stg, in0=stg, in1=rec_b, op=OP.mult)
            nc.sync.dma_start(o_d[:, g4], stage[:, g4])

    nc.compile()
    return nc


_nc_cache = None


def _get_nc():
    global _nc_cache
    if _nc_cache is None:
        _nc_cache = build_kernel()
    return _nc_cache


def _core_pairs(x, core):
    flat = x.transpose(0, 2, 1, 3).reshape(N * H, L, D)
    return flat[2 * core:2 * core + 2]          # (2, L, D) fp32


def _t_layout(xc):
    # (2, L, D) -> (128, 2048) [(64p + d), (128c + i)]
    return np.ascontiguousarray(
        xc.reshape(PAIRS, NCH, C, D).transpose(0, 3, 1, 2).reshape(C, W)
    ).astype(np.float16)


def _v_layout(xc):
    # (2, L, D) -> (128, 2080) [i, (c, p, dv)] with ones at dv=64
    v4 = xc.reshape(PAIRS, NCH, C, D).transpose(2, 1, 0, 3)  # (C, NCH, PAIRS, D)
    vaug = np.concatenate(
        [v4, np.ones((C, NCH, PAIRS, 1), v4.dtype)], axis=-1)
    return np.ascontiguousarray(vaug.reshape(C, VW)).astype(np.float16)


def make_in_maps(queries, keys, values):
    in_maps = []
    for core in range(8):
        in_maps.append({
            "qT": _t_layout(_core_pairs(queries, core)),
            "kT": _t_layout(_core_pairs(keys, core)),
            "v": _v_layout(_core_pairs(values, core)),
        })
    return in_maps


def _unpack_out(o):
    # (128, 2048) [i', (c, p, d)] -> (PAIRS, L, D)
    return o.reshape(C, NCH, PAIRS, D).transpose(2, 1, 0, 3).reshape(PAIRS, L, D)


def kernel(queries, keys, values):
    nc = _get_nc()
    in_maps = make_in_maps(queries, keys, values)
    res = run_bass_kernel_spmd(nc, in_maps, core_ids=list(range(8)))
    out = np.zeros((N, L, H, D), np.float32)
    for core in range(8):
        oc = _unpack_out(res.results[core]["o"].astype(np.float32))
        for p in range(PAIRS):
            flat = 2 * core + p
            out[flat // H, :, flat % H, :] = oc[p]
    return out
